# revision 11
# baseline (speedup 1.0000x reference)
"""Trainium2 Bass kernel for AvgClicksPoolingInitializer (segment_reduce).

Reference semantics (per batch b):
  for each feature level l (128^2, 64^2, 32^2, 16^2 spatial):
    m   = bilinear_resize(scribbles[b], (h_l, w_l))          # [I, h, w]
    sel = m > 0.5
    s   = einsum('ip,cp->ic', sel, f_l)                      # masked sum
    cnt = sel.sum(-1)
    mean_l = s / max(cnt, 1)   (fallback gather never taken for these inputs)
  out[b] = mean(mean_l over levels)                          # [I, C]

Key identity used on-device: bilinear downsample by integer factor s with
half-pixel centers and antialias=False samples exactly two taps per axis with
weights (0.5, 0.5) at offset o = s/2 - 1.  Hence
    4*m[r, c] = (x[s*r+o, s*c+o] + x[s*r+o+1, s*c+o]) +
                (x[s*r+o, s*c+o+1] + x[s*r+o+1, s*c+o+1])
and m > 0.5 iff the block sum > 2.0.

Host staging is layout/dtype only (gather + cast, zero arithmetic):
  - scrq: for every level/mask/output-pixel, the exact 4 scribble taps of the
    2x2 block, pre-gathered to [q(128), i, k, 4] fp16 so the threshold's
    output IS the stationary sel layout (q = within-chunk pixel index,
    k = 128-pixel chunk).  Only 2/s of each scribble row/col is ever used, so
    this is 2.78 MB/core vs 16.8 MB of raw scribbles.
  - ft: feature levels transposed to [pixel, 257] fp16 with a literal 1.0 in
    column 256 (the cnt column), tiled per 8-chunk stream tile so every DMA
    is one fully contiguous HBM block.

Precision: fp16 staging is a dtype cast; all arithmetic runs on device.  The
two pair-sum adds run f32 on fp16 inputs — exact (4-term fp16 sums fit f32),
so sel deviates from the f32 reference only where fp16 INPUT rounding moves a
block sum across 2.0: measured offline, 327 flips, rel l2 1.77e-3 (gate
2e-2).  The matmul accumulates fp16 products exactly into f32 PSUM.

Sharding: data-parallel over batch B=8 across the 8 NeuronCores (1 each).

Per-core device pipeline (levels smallest-first, each level's sel build
software-pipelined one level ahead of its matmul stream):
  1. One or two DMAs pull the level's tap block; two fused f32 DVE adds + one
     fp16 threshold write sel straight into the stationary tile. No
     transposes, no PSUM staging.
  2. ft streams in 8-chunk fp16 tiles; one fp16 matmul per 128-pixel chunk
     with sel stationary [128,16] and moving [128,257] (ones column -> cnt),
     accumulating (sum, cnt) per level in f32 PSUM.
  3. Per-level fused finalize: rec = 0.25/max(cnt,1), multiply-accumulate
     into the running 4-level average; DMA out [16,256] f32.

Cost-model roofline: ~13.9 MB/core of DMA at 360 B/ns => ~39 us transfer;
PE (~170 fp16 matmuls) and DVE (~13 us) overlap under it.
"""

import os
import sys

import numpy as np

for _p in ("/opt/trn_rl_repo", "/root/.axon_site/_ro/trn_rl_repo"):
    if os.path.isdir(_p) and _p not in sys.path:
        sys.path.insert(0, _p)

import concourse.bass as bass
import concourse.mybir as mybir
from concourse.bass_utils import run_bass_kernel_spmd
from concourse.tile import TileContext

F32 = mybir.dt.float32
F16 = mybir.dt.float16
F8 = mybir.dt.float8e4

B, I, C = 8, 16, 256
CW = C + 1  # feature row + ones column (fp16 levels)
CW8 = 272  # fp8 levels: +15 zero pad so DoubleRow halves are 16B-aligned
# (stride s, out hw, tap offset o, 128-pixel chunks nk)
LEVELS = [
    (4, 128, 1, 128),
    (8, 64, 3, 32),
    (16, 32, 7, 8),
    (32, 16, 15, 2),
]
# L0/L1 features+sel ride fp8e4m3 with DoubleRow matmuls (error measured
# offline: rel 2.20e-3 incl. the fp16 scribble flips); L2/L3 stay fp16.
FT_DT = {0: F8, 1: F8, 2: F16, 3: F16}
CWL = {l: (CW8 if FT_DT[l] == F8 else CW) for l in range(4)}
P_TOTAL = sum(hw * hw for _, hw, _, _ in LEVELS)  # 21760
N_CHUNKS = P_TOTAL // 128  # 170
# chunks per streamed ft tile (~526/514 KiB DMAs)
FT_TILE_CHUNKS = {0: 16, 1: 16, 2: 8, 3: 8}
# Process levels smallest-first so the PE gets sel masks + feature data within
# a few us of launch instead of waiting out all scribble DMAs.
STREAM_ORDER = (3, 2, 1, 0)
# sel builds are split into k-ranges (one DMA + add/add/threshold chain per
# split) so stationary sel production pipelines with the matmul stream
# instead of forming one long serial DVE chain.
SCR_SPLITS = {0: 8, 1: 2, 2: 1, 3: 1}
SCRQ_SIZES = {l: 128 * I * LEVELS[l][3] * 4 for l in range(4)}
SCRQ_OFFS = {}
_off = 0
for _l in STREAM_ORDER:
    SCRQ_OFFS[_l] = _off
    _off += SCRQ_SIZES[_l]
SCRQ_TOTAL = _off
# per-level chunk offsets within the fp8 / fp16 ft streams
FT8_OFFS, FT16_OFFS = {}, {}
_o8 = _o16 = 0
for _l in STREAM_ORDER:
    if FT_DT[_l] == F8:
        FT8_OFFS[_l] = _o8
        _o8 += LEVELS[_l][3]
    else:
        FT16_OFFS[_l] = _o16
        _o16 += LEVELS[_l][3]
FT8_CHUNKS, FT16_CHUNKS = _o8, _o16


def _ft_tile_sizes(l):
    """Chunk counts of level l's stream tiles — shared by host staging and
    the device stream so both agree on the partition-major block layout."""
    nk = LEVELS[l][3]
    sizes = []
    k = 0
    while k < nk:
        n = min(FT_TILE_CHUNKS[l], nk - k)
        if l == 0 and nk - k == 16:
            n = 8  # split L0's last tile so the tail drain is short
        sizes.append(n)
        k += n
    return sizes


def _split_excess_waits(nc: bass.Bass, cap: int = 1) -> int:
    """The pinned walrus codegen rejects instructions carrying more than one
    semaphore wait (setupSyncWait: "Too many sync wait commands").  Hoist
    excess waits onto injected same-engine NOPs placed immediately before the
    instruction — engine queues execute in order, so semantics are unchanged.
    """
    n_split = 0
    for bb in nc.m.functions[0].blocks:
        out = []
        for inst in bb.instructions:
            si = getattr(inst, "sync_info", None)
            if si is not None and si.on_wait and len(si.on_wait) > cap:
                waits = list(si.on_wait)
                keep, excess = waits[:cap], waits[cap:]
                for i in range(0, len(excess), cap):
                    n_split += 1
                    nop = mybir.InstNoOp(
                        name=f"{inst.name}-wsp{i}",
                        sync_info=mybir.SyncInfo(
                            on_wait=excess[i:i + cap], on_update=[]),
                        bass_nofuse=True,
                        engine=inst.engine,
                    )
                    nc.register_instruction(nop, overwrite=True)
                    out.append(nop)
                inst.sync_info = mybir.SyncInfo(
                    on_wait=keep, on_update=list(si.on_update))
            out.append(inst)
        bb.instructions = out
    return n_split


def build_program(n_cores: int = 8, *, ftp_bufs: int = 10,
                  workp_bufs: int = 2) -> bass.Bass:
    nc = bass.Bass("TRN2", target_bir_lowering=False, debug=False,
                   num_devices=n_cores)

    ft8 = nc.dram_tensor("ft8", [FT8_CHUNKS * 128 * CW8], F8,
                         kind="ExternalInput").ap()
    ft16 = nc.dram_tensor("ft16", [FT16_CHUNKS * 128 * CW], F16,
                          kind="ExternalInput").ap()
    scrq = nc.dram_tensor("scrq", [SCRQ_TOTAL], F16,
                          kind="ExternalInput").ap()
    out = nc.dram_tensor("out", [I, C], F32, kind="ExternalOutput").ap()

    with TileContext(nc) as tc:
        with (
            tc.sbuf_pool(name="selp", bufs=1) as selp,
            tc.sbuf_pool(name="workp", bufs=workp_bufs) as workp,
            tc.sbuf_pool(name="ftp", bufs=ftp_bufs) as ftp,
            tc.sbuf_pool(name="finp", bufs=1) as finp,
            tc.psum_pool(name="accp", bufs=1) as accp,
        ):
            _emit_body(nc, tc, ft8, ft16, scrq, out, selp, workp, ftp,
                       finp, accp)

    _split_excess_waits(nc)
    return nc


def _emit_resize(nc, workp, scrq, S, l):
    """Generator (one yield per k-range split): build sel for level l.

    The staged tap block is [q(128), (k, i, cx, rx)] fp16; per split, one DMA
    plus two fused f32 adds (rows first, matching the resize identity) and an
    fp16/fp8 threshold write that k-range of the stationary sel tile S[l]
    directly.  Splitting keeps each chain short so sel production pipelines
    with the previous level's matmul stream.
    """
    ik = I * LEVELS[l][3]
    src = scrq[SCRQ_OFFS[l]:SCRQ_OFFS[l] + SCRQ_SIZES[l]].rearrange(
        "(q f) -> q f", q=128)
    A = workp.tile([128, ik * 4], F16, tag=f"A{l}", name=f"A{l}", bufs=1)
    nsp = SCR_SPLITS[l]
    n = ik // nsp  # sel elements per split (k-major: contiguous k-range)
    for sp in range(nsp):
        nc.sync.dma_start(out=A[:, sp * 4 * n:(sp + 1) * 4 * n],
                          in_=src[:, sp * 4 * n:(sp + 1) * 4 * n])
        Av = A[:, sp * 4 * n:(sp + 1) * 4 * n].rearrange(
            "q (m rx) -> q m rx", rx=2)
        R = workp.tile([128, 2 * n], F32, tag=f"R{l}", name=f"R{l}_{sp}",
                       bufs=2)
        nc.vector.tensor_add(R[:, :], Av[:, :, 0], Av[:, :, 1])
        Rv = R.rearrange("q (m cx) -> q m cx", cx=2)
        S4 = workp.tile([128, n], F32, tag=f"S4_{l}", name=f"S4_{l}_{sp}",
                        bufs=2)
        nc.vector.tensor_add(S4[:, :], Rv[:, :, 0], Rv[:, :, 1])
        nc.vector.tensor_scalar(
            S[l][:, sp * n:(sp + 1) * n], S4[:, :], 2.0, None,
            op0=mybir.AluOpType.is_gt
        )
        yield


def _emit_stream_level(nc, ftp, ft, S, acc, l, ft_off):
    """Generator: one yield per streamed ft tile + its matmuls.

    fp8 levels run DoubleRow matmuls: lhsT/rhs carry two consecutive chunks
    block-concatenated along the free dim (S free layout is (k, i), the ft
    tile is chunk-major), accumulating both chunks in one instruction."""
    nk = LEVELS[l][3]
    dt = FT_DT[l]
    dr = dt == F8  # DoubleRow
    cw = CWL[l]
    tile_chunks = FT_TILE_CHUNKS[l]
    k = 0
    for n in _ft_tile_sizes(l):
        g0 = ft_off + k
        FT = ftp.tile([128, n * cw], dt, tag=f"FT{'8' if dr else '16'}",
                      name=f"FT{l}_{g0}",
                      padded_shape=[128, tile_chunks * cw])
        src = ft[128 * cw * g0:128 * cw * (g0 + n)].rearrange(
            "(p cx) -> p cx", p=128)
        nc.sync.dma_start(out=FT[:, :], in_=src)
        step = 2 if dr else 1
        for j in range(0, n, step):
            if dr:
                lhsT = S[l][:, (k + j) * I:(k + j + 2) * I].rearrange(
                    "q (two i) -> q two i", two=2)
                rhs = FT[:, j * cw:(j + 2) * cw].rearrange(
                    "p (two x) -> p two x", two=2)
            else:
                lhsT = S[l][:, (k + j) * I:(k + j + 1) * I]
                rhs = FT[:, j * cw:(j + 1) * cw]
            nc.tensor.matmul(
                acc[l][:, :],
                lhsT=lhsT,
                rhs=rhs,
                start=(k + j == 0),
                stop=(k + j + step == nk),
                perf_mode=(mybir.MatmulPerfMode.DoubleRow if dr else None),
            )
        k += n
        yield


def _emit_finalize_level(nc, finp, acc, l, prev_msum):
    """rec = 0.25/max(cnt,1) (exact: x4 is a power-of-2 scale), then fused
    multiply-accumulate into the running level average."""
    cnt4 = finp.tile([I, 1], F32, name=f"cnt4_{l}", tag=f"cnt4_{l}")
    nc.vector.tensor_scalar(
        cnt4[:, :], acc[l][:, C:C + 1], 1.0, 4.0,
        op0=mybir.AluOpType.max, op1=mybir.AluOpType.mult)
    rec = finp.tile([I, 1], F32, name=f"rec{l}", tag=f"rec{l}")
    nc.vector.reciprocal(rec[:, :], cnt4[:, :])
    msum = finp.tile([I, C], F32, name=f"msum{l}", tag=f"msum{l}")
    if prev_msum is None:
        nc.vector.tensor_scalar_mul(
            msum[:, :], acc[l][:, 0:C], rec[:, 0:1])
    else:
        nc.vector.scalar_tensor_tensor(
            out=msum[:, :], in0=acc[l][:, 0:C], scalar=rec[:, 0:1],
            in1=prev_msum[:, :],
            op0=mybir.AluOpType.mult, op1=mybir.AluOpType.add)
    return msum


def _drain(gen):
    if gen is not None:
        for _ in gen:
            pass


def _emit_body(nc, tc, ft8, ft16, scrq, out, selp, workp, ftp, finp, accp):
    # Persistent stationary sel tiles: S[l][q, k*I + i] where q = dr*hw + c
    # is the within-chunk partition index (pixel p = 128*k + q, r = k*ndr+dr).
    S = [
        selp.tile([128, I * nk], FT_DT[l], name=f"selT{l}", tag=f"selT{l}")
        for l, (_, _, _, nk) in enumerate(LEVELS)
    ]
    acc = [
        accp.tile([I, CWL[l]], F32, name=f"acc{l}", tag=f"acc{l}")
        for l in range(len(LEVELS))
    ]


    # Software pipeline: level l's sel build is fully emitted before level
    # l's stream; the NEXT level's scr DMA + sel build interleave into the
    # current level's stream at ft-tile granularity.
    prev_msum = None
    _drain(_emit_resize(nc, workp, scrq, S, STREAM_ORDER[0]))
    for idx, l in enumerate(STREAM_ORDER):
        nxt_gen = (_emit_resize(nc, workp, scrq, S, STREAM_ORDER[idx + 1])
                   if idx + 1 < len(STREAM_ORDER) else None)
        ftl = ft8 if FT_DT[l] == F8 else ft16
        offl = FT8_OFFS[l] if FT_DT[l] == F8 else FT16_OFFS[l]
        for _ in _emit_stream_level(nc, ftp, ftl, S, acc, l, offl):
            if nxt_gen is not None:
                next(nxt_gen, None)
        _drain(nxt_gen)
        prev_msum = _emit_finalize_level(nc, finp, acc, l, prev_msum)

    nc.sync.dma_start(out=out[:, :], in_=prev_msum[:, :])


_PROGRAM_CACHE: dict[int, bass.Bass] = {}


def _get_program(n_cores: int = 8) -> bass.Bass:
    if n_cores not in _PROGRAM_CACHE:
        _PROGRAM_CACHE[n_cores] = build_program(n_cores)
    return _PROGRAM_CACHE[n_cores]


def _stage_inputs(feat0, feat1, feat2, feat3, scribbles):
    """Per-core input maps: batch-shard, fp16-cast, transpose features to
    [P, 257] (ones column baked in) and tap-gather the scribbles.  Layout and
    dtype staging only — all arithmetic runs on device."""
    import ml_dtypes
    E4 = ml_dtypes.float8_e4m3fn
    feats = [np.asarray(f, dtype=np.float32) for f in
             (feat0, feat1, feat2, feat3)]
    scribbles = np.asarray(scribbles, dtype=np.float32)
    in_maps = []
    for b in range(B):
        # ft: levels concatenated in STREAM_ORDER into an fp8 stream (L0/L1)
        # and an fp16 stream (L2/L3), [P_l, 257] each, re-tiled so every
        # stream tile is one contiguous [p, c_tile, 257] block.
        blocks8, blocks16 = [], []
        for l in STREAM_ORDER:
            nk = LEVELS[l][3]
            np_dt = E4 if FT_DT[l] == F8 else np.float16
            cw = CWL[l]
            ftl = feats[l][b].reshape(C, -1).T.astype(np_dt)  # [P_l, C]
            ext = np.concatenate(
                [ftl, np.ones((ftl.shape[0], 1), dtype=np_dt),
                 np.zeros((ftl.shape[0], cw - CW), dtype=np_dt)], axis=1)
            k = 0
            for n in _ft_tile_sizes(l):
                blk = ext[128 * k:128 * (k + n)].reshape(n, 128, cw)
                (blocks8 if FT_DT[l] == F8 else blocks16).append(
                    np.ascontiguousarray(blk.transpose(1, 0, 2)).ravel())
                k += n
        ft8_staged = np.concatenate(blocks8)
        ft16_staged = np.concatenate(blocks16)
        assert ft8_staged.shape == (FT8_CHUNKS * 128 * CW8,)
        assert ft16_staged.shape == (FT16_CHUNKS * 128 * CW,)

        # scrq: per level the 4 taps of every 2x2 block, [q, i, k, cx, rx]
        # where q = dr*hw + c, chunk k, and the adds collapse rx then cx.
        scr_blocks = []
        scr_b = scribbles[b]  # [I, 512, 512] f32
        for l in STREAM_ORDER:
            s, hw, o, nk = LEVELS[l]
            ndr = 128 // hw
            rr = s * np.arange(hw) + o
            cc = s * np.arange(hw) + o
            t00 = scr_b[:, rr][:, :, cc]
            t10 = scr_b[:, rr + 1][:, :, cc]
            t01 = scr_b[:, rr][:, :, cc + 1]
            t11 = scr_b[:, rr + 1][:, :, cc + 1]
            T4 = np.stack([t00, t10, t01, t11], axis=-1)  # [I, r, c, (cx,rx)]
            T4 = T4.reshape(I, nk, ndr, hw, 4)            # r -> (k, dr)
            Aq = T4.transpose(2, 3, 1, 0, 4)              # [dr, c, k, i, 4]
            scr_blocks.append(
                np.ascontiguousarray(Aq).astype(np.float16).ravel())
        scr_staged = np.concatenate(scr_blocks)
        assert scr_staged.shape == (SCRQ_TOTAL,)

        in_maps.append({"ft8": ft8_staged, "ft16": ft16_staged,
                        "scrq": scr_staged})
    return in_maps


def run(feat0, feat1, feat2, feat3, scribbles, trace: bool = False,
        **spmd_kwargs):
    nc = _get_program(B)
    in_maps = _stage_inputs(feat0, feat1, feat2, feat3, scribbles)
    res = run_bass_kernel_spmd(
        nc, in_maps, core_ids=list(range(B)), trace=trace, **spmd_kwargs
    )
    out = np.stack([res.results[b]["out"] for b in range(B)], axis=0)
    return out.astype(np.float32), res


def kernel(feat0, feat1, feat2, feat3, scribbles):
    out, _ = run(feat0, feat1, feat2, feat3, scribbles)
    return out


# revision 13
# speedup vs baseline: 1.1413x; 1.1413x over previous
"""Trainium2 Bass kernel for AvgClicksPoolingInitializer (segment_reduce).

Reference semantics (per batch b):
  for each feature level l (128^2, 64^2, 32^2, 16^2 spatial):
    m   = bilinear_resize(scribbles[b], (h_l, w_l))          # [I, h, w]
    sel = m > 0.5
    s   = einsum('ip,cp->ic', sel, f_l)                      # masked sum
    cnt = sel.sum(-1)
    mean_l = s / max(cnt, 1)   (fallback gather never taken for these inputs)
  out[b] = mean(mean_l over levels)                          # [I, C]

Key identity used on-device: bilinear downsample by integer factor s with
half-pixel centers and antialias=False samples exactly two taps per axis with
weights (0.5, 0.5) at offset o = s/2 - 1.  Hence
    4*m[r, c] = (x[s*r+o, s*c+o] + x[s*r+o+1, s*c+o]) +
                (x[s*r+o, s*c+o+1] + x[s*r+o+1, s*c+o+1])
and m > 0.5 iff the block sum > 2.0.

Host staging is layout/dtype only (gather + cast, zero arithmetic):
  - scrq: for every level/mask/output-pixel, the exact 4 scribble taps of the
    2x2 block, pre-gathered to [q(128), i, k, 4] fp16 so the threshold's
    output IS the stationary sel layout (q = within-chunk pixel index,
    k = 128-pixel chunk).  Only 2/s of each scribble row/col is ever used, so
    this is 2.78 MB/core vs 16.8 MB of raw scribbles.
  - ft: feature levels transposed to [pixel, 257] fp16 with a literal 1.0 in
    column 256 (the cnt column), tiled per 8-chunk stream tile so every DMA
    is one fully contiguous HBM block.

Precision: fp16 staging is a dtype cast; all arithmetic runs on device.  The
two pair-sum adds run f32 on fp16 inputs — exact (4-term fp16 sums fit f32),
so sel deviates from the f32 reference only where fp16 INPUT rounding moves a
block sum across 2.0: measured offline, 327 flips, rel l2 1.77e-3 (gate
2e-2).  The matmul accumulates fp16 products exactly into f32 PSUM.

Sharding: data-parallel over batch B=8 across the 8 NeuronCores (1 each).

Per-core device pipeline (levels smallest-first, each level's sel build
software-pipelined one level ahead of its matmul stream):
  1. One or two DMAs pull the level's tap block; two fused f32 DVE adds + one
     fp16 threshold write sel straight into the stationary tile. No
     transposes, no PSUM staging.
  2. ft streams in 8-chunk fp16 tiles; one fp16 matmul per 128-pixel chunk
     with sel stationary [128,16] and moving [128,257] (ones column -> cnt),
     accumulating (sum, cnt) per level in f32 PSUM.
  3. Per-level fused finalize: rec = 0.25/max(cnt,1), multiply-accumulate
     into the running 4-level average; DMA out [16,256] f32.

Cost-model roofline: ~13.9 MB/core of DMA at 360 B/ns => ~39 us transfer;
PE (~170 fp16 matmuls) and DVE (~13 us) overlap under it.
"""

import os
import sys

import numpy as np

for _p in ("/opt/trn_rl_repo", "/root/.axon_site/_ro/trn_rl_repo"):
    if os.path.isdir(_p) and _p not in sys.path:
        sys.path.insert(0, _p)

import concourse.bass as bass
import concourse.mybir as mybir
from concourse.bass_utils import run_bass_kernel_spmd
from concourse.tile import TileContext

F32 = mybir.dt.float32
F16 = mybir.dt.float16
F8 = mybir.dt.float8e4

B, I, C = 8, 16, 256
CW = C + 1  # feature row + ones column (fp16 levels)
CW8 = 272  # fp8 levels: +15 zero pad so DoubleRow halves are 16B-aligned
# (stride s, out hw, tap offset o, 128-pixel chunks nk)
LEVELS = [
    (4, 128, 1, 128),
    (8, 64, 3, 32),
    (16, 32, 7, 8),
    (32, 16, 15, 2),
]
# L0/L1 features+sel ride fp8e4m3 with DoubleRow matmuls (error measured
# offline: rel 2.20e-3 incl. the fp16 scribble flips); L2/L3 stay fp16.
FT_DT = {0: F8, 1: F8, 2: F16, 3: F16}
CWL = {l: (CW8 if FT_DT[l] == F8 else CW) for l in range(4)}
P_TOTAL = sum(hw * hw for _, hw, _, _ in LEVELS)  # 21760
N_CHUNKS = P_TOTAL // 128  # 170
# chunks per streamed ft tile (~526/514 KiB DMAs)
FT_TILE_CHUNKS = {0: 16, 1: 16, 2: 8, 3: 8}
# Process levels smallest-first so the PE gets sel masks + feature data within
# a few us of launch instead of waiting out all scribble DMAs.
STREAM_ORDER = (3, 2, 1, 0)
# sel builds are split into k-ranges (one DMA + add/add/threshold chain per
# split) so stationary sel production pipelines with the matmul stream
# instead of forming one long serial DVE chain.
SCR_SPLITS = {0: 8, 1: 2, 2: 1, 3: 1}
# L0 taps ride fp8e4m3 (their sel flips cost the least: err ~ 1/sqrt(P_l));
# L1-L3 taps stay fp16.  Measured offline: rel 4.49e-3 total.
SCR_DT = {0: "f8", 1: "f16", 2: "f16", 3: "f16"}
SCRQ_SIZES = {l: 128 * I * LEVELS[l][3] * 4 for l in range(4)}
SCRQ_OFFS = {}          # offsets within the fp16 tap stream (levels 3,2,1)
_off = 0
for _l in STREAM_ORDER:
    if SCR_DT[_l] == "f16":
        SCRQ_OFFS[_l] = _off
        _off += SCRQ_SIZES[_l]
SCRQ16_TOTAL = _off
SCRQ8_TOTAL = SCRQ_SIZES[0]
# per-level chunk offsets within the fp8 / fp16 ft streams
FT8_OFFS, FT16_OFFS = {}, {}
_o8 = _o16 = 0
for _l in STREAM_ORDER:
    if FT_DT[_l] == F8:
        FT8_OFFS[_l] = _o8
        _o8 += LEVELS[_l][3]
    else:
        FT16_OFFS[_l] = _o16
        _o16 += LEVELS[_l][3]
FT8_CHUNKS, FT16_CHUNKS = _o8, _o16


def _ft_tile_sizes(l):
    """Chunk counts of level l's stream tiles — shared by host staging and
    the device stream so both agree on the partition-major block layout."""
    nk = LEVELS[l][3]
    sizes = []
    k = 0
    while k < nk:
        n = min(FT_TILE_CHUNKS[l], nk - k)
        if l == 0 and nk - k == 16:
            n = 8  # split L0's last tile so the tail drain is short
        sizes.append(n)
        k += n
    return sizes


def _split_excess_waits(nc: bass.Bass, cap: int = 1) -> int:
    """The pinned walrus codegen rejects instructions carrying more than one
    semaphore wait (setupSyncWait: "Too many sync wait commands").  Hoist
    excess waits onto injected same-engine NOPs placed immediately before the
    instruction — engine queues execute in order, so semantics are unchanged.
    """
    n_split = 0
    for bb in nc.m.functions[0].blocks:
        out = []
        for inst in bb.instructions:
            si = getattr(inst, "sync_info", None)
            if si is not None and si.on_wait and len(si.on_wait) > cap:
                waits = list(si.on_wait)
                keep, excess = waits[:cap], waits[cap:]
                for i in range(0, len(excess), cap):
                    n_split += 1
                    nop = mybir.InstNoOp(
                        name=f"{inst.name}-wsp{i}",
                        sync_info=mybir.SyncInfo(
                            on_wait=excess[i:i + cap], on_update=[]),
                        bass_nofuse=True,
                        engine=inst.engine,
                    )
                    nc.register_instruction(nop, overwrite=True)
                    out.append(nop)
                inst.sync_info = mybir.SyncInfo(
                    on_wait=keep, on_update=list(si.on_update))
            out.append(inst)
        bb.instructions = out
    return n_split


def build_program(n_cores: int = 8, *, ftp_bufs: int = 10,
                  workp_bufs: int = 2) -> bass.Bass:
    nc = bass.Bass("TRN2", target_bir_lowering=False, debug=False,
                   num_devices=n_cores)

    ft8 = nc.dram_tensor("ft8", [FT8_CHUNKS * 128 * CW8], F8,
                         kind="ExternalInput").ap()
    ft16 = nc.dram_tensor("ft16", [FT16_CHUNKS * 128 * CW], F16,
                          kind="ExternalInput").ap()
    scrq8 = nc.dram_tensor("scrq8", [SCRQ8_TOTAL], F8,
                           kind="ExternalInput").ap()
    scrq16 = nc.dram_tensor("scrq16", [SCRQ16_TOTAL], F16,
                            kind="ExternalInput").ap()
    out = nc.dram_tensor("out", [I, C], F32, kind="ExternalOutput").ap()

    with TileContext(nc) as tc:
        with (
            tc.sbuf_pool(name="selp", bufs=1) as selp,
            tc.sbuf_pool(name="workp", bufs=workp_bufs) as workp,
            tc.sbuf_pool(name="ftp", bufs=ftp_bufs) as ftp,
            tc.sbuf_pool(name="finp", bufs=1) as finp,
            tc.psum_pool(name="accp", bufs=1) as accp,
        ):
            _emit_body(nc, tc, ft8, ft16, scrq8, scrq16, out, selp, workp,
                       ftp, finp, accp)

    _split_excess_waits(nc)
    return nc


def _sel_chain(nc, workp, Aslice, S, l, sp, n, s_off):
    """Two fused f32 adds (rows first, matching the resize identity) and a
    threshold writing sel elements [s_off, s_off+n) of S[l]."""
    Av = Aslice.rearrange("q (m rx) -> q m rx", rx=2)
    R = workp.tile([128, 2 * n], F32, tag=f"R{l}", name=f"R{l}_{sp}",
                   bufs=2)
    nc.vector.tensor_add(R[:, :], Av[:, :, 0], Av[:, :, 1])
    Rv = R.rearrange("q (m cx) -> q m cx", cx=2)
    S4 = workp.tile([128, n], F32, tag=f"S4_{l}", name=f"S4_{l}_{sp}",
                    bufs=2)
    nc.vector.tensor_add(S4[:, :], Rv[:, :, 0], Rv[:, :, 1])
    nc.vector.tensor_scalar(
        S[l][:, s_off:s_off + n], S4[:, :], 2.0, None,
        op0=mybir.AluOpType.is_gt
    )


def _emit_resize(nc, workp, scrq_ap, ap_off, S, l):
    """Generator (one yield per k-range split): build sel for level l.

    The staged tap block is [q(128), (k, i, cx, rx)]; per split, one DMA plus
    the sel chain covering that k-range.  Splitting keeps each chain short so
    sel production pipelines with the matmul stream.
    """
    ik = I * LEVELS[l][3]
    dt = F8 if SCR_DT[l] == "f8" else F16
    src = scrq_ap[ap_off:ap_off + SCRQ_SIZES[l]].rearrange(
        "(q f) -> q f", q=128)
    A = workp.tile([128, ik * 4], dt, tag=f"A{l}", name=f"A{l}", bufs=1)
    nsp = SCR_SPLITS[l]
    n = ik // nsp  # sel elements per split (k-major: contiguous k-range)
    for sp in range(nsp):
        nc.sync.dma_start(out=A[:, sp * 4 * n:(sp + 1) * 4 * n],
                          in_=src[:, sp * 4 * n:(sp + 1) * 4 * n])
        _sel_chain(nc, workp, A[:, sp * 4 * n:(sp + 1) * 4 * n], S, l,
                   sp, n, sp * n)
        yield


def _emit_resize_group32(nc, workp, scrq16, S):
    """Levels 3+2 sel build: their fp16 tap blocks are adjacent in scrq16,
    so one DMA feeds both levels' chains (saves an HWDGE turnaround in the
    prologue where DMA issue rate, not bandwidth, is the limiter)."""
    ik3, ik2 = I * LEVELS[3][3], I * LEVELS[2][3]
    tot = SCRQ_SIZES[3] + SCRQ_SIZES[2]
    src = scrq16[SCRQ_OFFS[3]:SCRQ_OFFS[3] + tot].rearrange(
        "(q f) -> q f", q=128)
    A = workp.tile([128, tot // 128], F16, tag="A32", name="A32", bufs=1)
    nc.sync.dma_start(out=A[:, :], in_=src)
    _sel_chain(nc, workp, A[:, 0:ik3 * 4], S, 3, 0, ik3, 0)
    _sel_chain(nc, workp, A[:, ik3 * 4:], S, 2, 0, ik2, 0)
    yield


def _emit_stream_pair32(nc, ftp, ft16, S, acc):
    """Combined L3+L2 stream: one 10-chunk fp16 tile, 10 plain matmuls."""
    FT = ftp.tile([128, 10 * CW], F16, tag="FT16", name="FT32")
    nc.sync.dma_start(
        out=FT[:, :],
        in_=ft16[0:128 * CW * 10].rearrange("(p cx) -> p cx", p=128))
    for l, j0, nkl in ((3, 0, 2), (2, 2, 8)):
        for j in range(nkl):
            nc.tensor.matmul(
                acc[l][:, :],
                lhsT=S[l][:, j * I:(j + 1) * I],
                rhs=FT[:, (j0 + j) * CW:(j0 + j + 1) * CW],
                start=(j == 0),
                stop=(j == nkl - 1),
            )
    yield


def _emit_stream_level(nc, ftp, ft, S, acc, l, ft_off):
    """Generator: one yield per streamed ft tile + its matmuls.

    fp8 levels run DoubleRow matmuls: lhsT/rhs carry two consecutive chunks
    block-concatenated along the free dim (S free layout is (k, i), the ft
    tile is chunk-major), accumulating both chunks in one instruction."""
    nk = LEVELS[l][3]
    dt = FT_DT[l]
    dr = dt == F8  # DoubleRow
    cw = CWL[l]
    tile_chunks = FT_TILE_CHUNKS[l]
    k = 0
    for n in _ft_tile_sizes(l):
        g0 = ft_off + k
        FT = ftp.tile([128, n * cw], dt, tag=f"FT{'8' if dr else '16'}",
                      name=f"FT{l}_{g0}",
                      padded_shape=[128, tile_chunks * cw])
        src = ft[128 * cw * g0:128 * cw * (g0 + n)].rearrange(
            "(p cx) -> p cx", p=128)
        nc.sync.dma_start(out=FT[:, :], in_=src)
        step = 2 if dr else 1
        for j in range(0, n, step):
            if dr:
                lhsT = S[l][:, (k + j) * I:(k + j + 2) * I].rearrange(
                    "q (two i) -> q two i", two=2)
                rhs = FT[:, j * cw:(j + 2) * cw].rearrange(
                    "p (two x) -> p two x", two=2)
            else:
                lhsT = S[l][:, (k + j) * I:(k + j + 1) * I]
                rhs = FT[:, j * cw:(j + 1) * cw]
            nc.tensor.matmul(
                acc[l][:, :],
                lhsT=lhsT,
                rhs=rhs,
                start=(k + j == 0),
                stop=(k + j + step == nk),
                perf_mode=(mybir.MatmulPerfMode.DoubleRow if dr else None),
            )
        k += n
        yield


def _emit_finalize_level(nc, finp, acc, l, prev_msum):
    """rec = 0.25/max(cnt,1) (exact: x4 is a power-of-2 scale), then fused
    multiply-accumulate into the running level average."""
    cnt4 = finp.tile([I, 1], F32, name=f"cnt4_{l}", tag=f"cnt4_{l}")
    nc.vector.tensor_scalar(
        cnt4[:, :], acc[l][:, C:C + 1], 1.0, 4.0,
        op0=mybir.AluOpType.max, op1=mybir.AluOpType.mult)
    rec = finp.tile([I, 1], F32, name=f"rec{l}", tag=f"rec{l}")
    nc.vector.reciprocal(rec[:, :], cnt4[:, :])
    msum = finp.tile([I, C], F32, name=f"msum{l}", tag=f"msum{l}")
    if prev_msum is None:
        nc.vector.tensor_scalar_mul(
            msum[:, :], acc[l][:, 0:C], rec[:, 0:1])
    else:
        nc.vector.scalar_tensor_tensor(
            out=msum[:, :], in0=acc[l][:, 0:C], scalar=rec[:, 0:1],
            in1=prev_msum[:, :],
            op0=mybir.AluOpType.mult, op1=mybir.AluOpType.add)
    return msum


def _drain(gen):
    if gen is not None:
        for _ in gen:
            pass


def _emit_body(nc, tc, ft8, ft16, scrq8, scrq16, out, selp, workp, ftp,
               finp, accp):
    # Persistent stationary sel tiles: S[l][q, k*I + i] where q = dr*hw + c
    # is the within-chunk partition index (pixel p = 128*k + q, r = k*ndr+dr).
    S = [
        selp.tile([128, I * nk], FT_DT[l], name=f"selT{l}", tag=f"selT{l}")
        for l, (_, _, _, nk) in enumerate(LEVELS)
    ]
    acc = [
        accp.tile([I, CWL[l]], F32, name=f"acc{l}", tag=f"acc{l}")
        for l in range(len(LEVELS))
    ]


    # Software pipeline: each sel build is emitted (in k-range splits) ahead
    # of the matmuls that consume it; the next stage's scr DMAs interleave
    # into the current stream at ft-tile granularity.
    prev_msum = None
    _drain(_emit_resize_group32(nc, workp, scrq16, S))
    res1 = _emit_resize(nc, workp, scrq16, SCRQ_OFFS[1], S, 1)
    for _ in _emit_stream_pair32(nc, ftp, ft16, S, acc):
        next(res1, None)
    _drain(res1)
    prev_msum = _emit_finalize_level(nc, finp, acc, 3, prev_msum)
    prev_msum = _emit_finalize_level(nc, finp, acc, 2, prev_msum)

    res0 = _emit_resize(nc, workp, scrq8, 0, S, 0)
    for _ in _emit_stream_level(nc, ftp, ft8, S, acc, 1, FT8_OFFS[1]):
        next(res0, None)
    prev_msum = _emit_finalize_level(nc, finp, acc, 1, prev_msum)

    # L0: advance the sel-split generator BEFORE each ft tile so every
    # split's threshold is emitted ahead of the matmuls that read it.
    stream0 = _emit_stream_level(nc, ftp, ft8, S, acc, 0, FT8_OFFS[0])
    while True:
        next(res0, None)
        try:
            next(stream0)
        except StopIteration:
            break
    prev_msum = _emit_finalize_level(nc, finp, acc, 0, prev_msum)

    nc.sync.dma_start(out=out[:, :], in_=prev_msum[:, :])


_PROGRAM_CACHE: dict[int, bass.Bass] = {}


def _get_program(n_cores: int = 8) -> bass.Bass:
    if n_cores not in _PROGRAM_CACHE:
        _PROGRAM_CACHE[n_cores] = build_program(n_cores)
    return _PROGRAM_CACHE[n_cores]


def _stage_inputs(feat0, feat1, feat2, feat3, scribbles):
    """Per-core input maps: batch-shard, fp16-cast, transpose features to
    [P, 257] (ones column baked in) and tap-gather the scribbles.  Layout and
    dtype staging only — all arithmetic runs on device."""
    import ml_dtypes
    E4 = ml_dtypes.float8_e4m3fn
    feats = [np.asarray(f, dtype=np.float32) for f in
             (feat0, feat1, feat2, feat3)]
    scribbles = np.asarray(scribbles, dtype=np.float32)
    in_maps = []
    for b in range(B):
        # ft: levels concatenated in STREAM_ORDER into an fp8 stream (L0/L1)
        # and an fp16 stream (L2/L3), [P_l, 257] each, re-tiled so every
        # stream tile is one contiguous [p, c_tile, 257] block.
        blocks8, blocks16 = [], []
        ext16 = []  # L3+L2 merged into one 10-chunk fp16 tile
        for l in STREAM_ORDER:
            nk = LEVELS[l][3]
            np_dt = E4 if FT_DT[l] == F8 else np.float16
            cw = CWL[l]
            ftl = feats[l][b].reshape(C, -1).T.astype(np_dt)  # [P_l, C]
            ext = np.concatenate(
                [ftl, np.ones((ftl.shape[0], 1), dtype=np_dt),
                 np.zeros((ftl.shape[0], cw - CW), dtype=np_dt)], axis=1)
            if FT_DT[l] == F16:
                ext16.append(ext)
                continue
            k = 0
            for n in _ft_tile_sizes(l):
                blk = ext[128 * k:128 * (k + n)].reshape(n, 128, cw)
                blocks8.append(
                    np.ascontiguousarray(blk.transpose(1, 0, 2)).ravel())
                k += n
        ext = np.concatenate(ext16, axis=0)  # [10*128, 257]
        blk = ext.reshape(10, 128, CW)
        blocks16.append(np.ascontiguousarray(blk.transpose(1, 0, 2)).ravel())
        ft8_staged = np.concatenate(blocks8)
        ft16_staged = np.concatenate(blocks16)
        assert ft8_staged.shape == (FT8_CHUNKS * 128 * CW8,)
        assert ft16_staged.shape == (FT16_CHUNKS * 128 * CW,)

        # scrq: per level the 4 taps of every 2x2 block, [q, k, i, cx, rx]
        # where q = dr*hw + c, chunk k, and the adds collapse rx then cx.
        # L0 goes to the fp8 stream, L1-L3 to the fp16 stream.
        blocks_s8, blocks_s16 = [], []
        scr_b = scribbles[b]  # [I, 512, 512] f32
        for l in STREAM_ORDER:
            s, hw, o, nk = LEVELS[l]
            ndr = 128 // hw
            rr = s * np.arange(hw) + o
            cc = s * np.arange(hw) + o
            t00 = scr_b[:, rr][:, :, cc]
            t10 = scr_b[:, rr + 1][:, :, cc]
            t01 = scr_b[:, rr][:, :, cc + 1]
            t11 = scr_b[:, rr + 1][:, :, cc + 1]
            T4 = np.stack([t00, t10, t01, t11], axis=-1)  # [I, r, c, (cx,rx)]
            T4 = T4.reshape(I, nk, ndr, hw, 4)            # r -> (k, dr)
            Aq = T4.transpose(2, 3, 1, 0, 4)              # [dr, c, k, i, 4]
            np_dt = E4 if SCR_DT[l] == "f8" else np.float16
            (blocks_s8 if SCR_DT[l] == "f8" else blocks_s16).append(
                np.ascontiguousarray(Aq).astype(np_dt).reshape(128, -1))
        # levels 3+2 share one DMA: store them jointly q-major
        scr8_staged = np.concatenate([blk.ravel() for blk in blocks_s8])
        scr16_staged = np.concatenate(
            [np.concatenate(blocks_s16[0:2], axis=1).ravel(),
             blocks_s16[2].ravel()])
        assert scr8_staged.shape == (SCRQ8_TOTAL,)
        assert scr16_staged.shape == (SCRQ16_TOTAL,)

        in_maps.append({"ft8": ft8_staged, "ft16": ft16_staged,
                        "scrq8": scr8_staged, "scrq16": scr16_staged})
    return in_maps


def run(feat0, feat1, feat2, feat3, scribbles, trace: bool = False,
        **spmd_kwargs):
    nc = _get_program(B)
    in_maps = _stage_inputs(feat0, feat1, feat2, feat3, scribbles)
    res = run_bass_kernel_spmd(
        nc, in_maps, core_ids=list(range(B)), trace=trace, **spmd_kwargs
    )
    out = np.stack([res.results[b]["out"] for b in range(B)], axis=0)
    return out.astype(np.float32), res


def kernel(feat0, feat1, feat2, feat3, scribbles):
    out, _ = run(feat0, feat1, feat2, feat3, scribbles)
    return out
